# revision 6
# baseline (speedup 1.0000x reference)
"""HGCN (2-layer hyperbolic GCN) Trainium2 Bass kernel, 8-way SPMD.

Sharding: nodes split into 8 contiguous shards (one per core); edges
partitioned by destination shard; per-layer tangent vectors exchanged with an
AllGather; per-edge gather of source tangent rows via indirect DMA; weighted
segment-sum done as PE matmuls against on-chip-built one-hot matrices.

All per-node norm-dependent scalars are computed in [128, NBLK] batches so the
scalar chains cost O(1) instructions per layer instead of O(tiles).
Transcendentals use only Ln/Exp/Square/Relu/Copy (one ACT table set:
natural_log_exp_and_others) -> no table switches. sqrt/tanh/artanh are built
from exp/ln.
"""

import sys

sys.path.insert(0, "/opt/trn_rl_repo")

import numpy as np

import concourse.bass as bass
import concourse.bacc as bacc
import concourse.tile as tile
from concourse import mybir
from concourse.masks import make_identity
from concourse.bass_utils import run_bass_kernel_spmd

AF = mybir.ActivationFunctionType
ALU = mybir.AluOpType
DT = mybir.dt

P = 128
MIN2 = 1e-30          # clamp for squared norms => norm clamp 1e-15
ACLIP = 1.0 - 1e-7    # artanh input clip
MAXN = 1.0 - 4e-3     # PROJ_EPS ball radius
E2MAX = 60.0          # clamp on exponent arg (tanh saturated long before)


# ----------------------------------------------------------------- helpers
def _batch_pool_tiles(es, tc, name, n, T):
    pool = es.enter_context(tc.tile_pool(name=name, bufs=1))
    return [pool.tile([P, T], DT.float32, name=f"{name}{i}") for i in range(n)]


def _sqrt_chain(nc, n2, t0, out_n, out_rn):
    """out_n = max(sqrt(n2),1e-15); out_rn = 1/out_n (via exp/ln)."""
    nc.vector.tensor_scalar(out=t0[:], in0=n2, scalar1=MIN2, scalar2=None,
                            op0=ALU.max)
    nc.scalar.activation(out=t0[:], in_=t0[:], func=AF.Ln)
    nc.scalar.activation(out=out_n[:], in_=t0[:], func=AF.Exp, scale=0.5)
    nc.scalar.activation(out=out_rn[:], in_=t0[:], func=AF.Exp, scale=-0.5)


def _tanh_pos(nc, x, t0, out):
    """out = tanh(x) for x>=0: 1 - 2/(exp(min(2x,60))+1). x may be clobbered."""
    nc.vector.tensor_scalar(out=t0[:], in0=x, scalar1=2.0, scalar2=E2MAX,
                            op0=ALU.mult, op1=ALU.min)
    nc.scalar.activation(out=t0[:], in_=t0[:], func=AF.Exp)
    nc.vector.tensor_scalar(out=t0[:], in0=t0[:], scalar1=1.0, scalar2=None,
                            op0=ALU.add)
    nc.vector.reciprocal(out=t0[:], in_=t0[:])
    nc.vector.tensor_scalar(out=out[:], in0=t0[:], scalar1=-2.0, scalar2=1.0,
                            op0=ALU.mult, op1=ALU.add)


def _artanh2(nc, z, t0, t1, out):
    """out = 2*artanh(z) = ln((1+z)/(1-z)), z in [0, 1)."""
    nc.vector.tensor_scalar(out=t0[:], in0=z, scalar1=1.0, scalar2=None,
                            op0=ALU.add)
    nc.vector.tensor_scalar(out=t1[:], in0=z, scalar1=-1.0, scalar2=1.0,
                            op0=ALU.mult, op1=ALU.add)
    nc.vector.reciprocal(out=t1[:], in_=t1[:])
    nc.vector.tensor_tensor(out=t0[:], in0=t0[:], in1=t1[:], op=ALU.mult)
    nc.scalar.activation(out=out[:], in_=t0[:], func=AF.Ln)


def _expmap_proj_chain(nc, n2, tt, out_s, out_hn):
    """From squared norms n2 of v: scale s so that h = v*s = proj(expmap0(v)),
    and out_hn = ||h|| (= min(max(tanh(n),1e-15),maxnorm)).
    tt: list of >=4 scratch [P,T] tiles."""
    t0, t1, t2, t3 = tt[:4]
    _sqrt_chain(nc, n2, t0, t1, t2)            # t1 = n, t2 = 1/n
    _tanh_pos(nc, t1[:], t0, t3)               # t3 = tanh(n)
    nc.vector.tensor_scalar(out=t0[:], in0=t3[:], scalar1=1e-15, scalar2=None,
                            op0=ALU.max)       # t0 = u = max(th,eps)
    nc.vector.tensor_scalar(out=out_hn[:], in0=t0[:], scalar1=MAXN,
                            scalar2=None, op0=ALU.min)   # hn = min(u,maxn)
    nc.vector.reciprocal(out=t0[:], in_=t0[:])           # 1/u
    nc.vector.tensor_tensor(out=t0[:], in0=out_hn[:], in1=t0[:], op=ALU.mult)
    # t0 = pf = hn/u ; s = tanh(n)/n * pf
    nc.vector.tensor_tensor(out=t3[:], in0=t3[:], in1=t2[:], op=ALU.mult)
    nc.vector.tensor_tensor(out=out_s[:], in0=t3[:], in1=t0[:], op=ALU.mult)


# ----------------------------------------------------------------- builder
def build_program(nc, NPAD, SHARD, NBLK, nb, coff, CTOT, y2s, ncores):
    """Trace the whole 2-layer HGCN SPMD program into nc."""
    f32 = DT.float32
    x_d = nc.dram_tensor("x", [SHARD, P], f32, kind="ExternalInput")
    wt1_d = nc.dram_tensor("wt1", [P, P], f32, kind="ExternalInput")
    wt2_d = nc.dram_tensor("wt2", [P, P], f32, kind="ExternalInput")
    hb1_d = nc.dram_tensor("hb1", [P, P], f32, kind="ExternalInput")
    hb2_d = nc.dram_tensor("hb2", [P, P], f32, kind="ExternalInput")
    midx_d = nc.dram_tensor("midx", [P, CTOT], DT.int32, kind="ExternalInput")
    mdst_d = nc.dram_tensor("mdst", [P, CTOT], f32, kind="ExternalInput")
    mew_d = nc.dram_tensor("mew", [P, CTOT], f32, kind="ExternalInput")
    out_d = nc.dram_tensor("out", [SHARD, P], f32, kind="ExternalOutput")

    from contextlib import ExitStack
    with tile.TileContext(nc) as tc, ExitStack() as es:
        # ---- persistent SBUF state
        consts = es.enter_context(tc.tile_pool(name="consts", bufs=1))
        ident = consts.tile([P, P], f32, name="ident")
        make_identity(nc, ident[:])
        iota_i = consts.tile([P, P], DT.int32, name="iota_i")
        nc.gpsimd.iota(iota_i[:], pattern=[[1, P]], base=0, channel_multiplier=0)
        iota_f = consts.tile([P, P], f32, name="iota_f")
        nc.vector.tensor_copy(out=iota_f[:], in_=iota_i[:])
        wt_sb = [consts.tile([P, P], f32, name=f"wt{l}") for l in range(2)]
        hb_sb = [consts.tile([P, P], f32, name=f"hbb{l}") for l in range(2)]
        nc.sync.dma_start(out=wt_sb[0][:], in_=wt1_d[:, :])
        nc.sync.dma_start(out=wt_sb[1][:], in_=wt2_d[:, :])
        nc.sync.dma_start(out=hb_sb[0][:], in_=hb1_d[:, :])
        nc.sync.dma_start(out=hb_sb[1][:], in_=hb2_d[:, :])
        midx_sb = consts.tile([P, CTOT], DT.int32, name="midx_sb")
        mdst_sb = consts.tile([P, CTOT], f32, name="mdst_sb")
        mew_sb = consts.tile([P, CTOT], f32, name="mew_sb")
        nc.sync.dma_start(out=midx_sb[:], in_=midx_d[:, :])
        nc.sync.dma_start(out=mdst_sb[:], in_=mdst_d[:, :])
        nc.sync.dma_start(out=mew_sb[:], in_=mew_d[:, :])

        big = es.enter_context(tc.tile_pool(name="big", bufs=1))
        V = big.tile([P, NBLK * P], f32, name="Vbuf")     # node tiles (col t)
        MX = big.tile([P, NBLK * P], f32, name="MXbuf")   # second big buffer

        def Vt(t):
            return V[:, t * P:(t + 1) * P]

        def Mt(t):
            return MX[:, t * P:(t + 1) * P]

        # batch scalar buffers
        nbt = _batch_pool_tiles(es, tc, "bt", 10, NBLK)
        (B0, B1, B2, B3, B4, B5, B6, B7, B8, B9) = nbt

        dram = es.enter_context(tc.tile_pool(name="dram", bufs=1, space="DRAM"))
        ag_in = [dram.tile([SHARD, P], f32, name=f"agin{l}") for l in range(2)]
        xt_full = [dram.tile([NPAD, P], f32, name=f"xtf{l}",
                             addr_space="Shared") for l in range(2)]

        work = es.enter_context(tc.tile_pool(name="work", bufs=3))
        psA = es.enter_context(tc.tile_pool(name="psA", bufs=2, space="PSUM"))
        psB = es.enter_context(tc.tile_pool(name="psB", bufs=2, space="PSUM"))
        psC = es.enter_context(tc.tile_pool(name="psC", bufs=2, space="PSUM"))
        gpool = es.enter_context(tc.tile_pool(name="gpool", bufs=2))
        nbmax = int(max(nb))
        rg = [list(range(ncores))]

        for l in range(2):
            # ---------------- phase A: per-node HypLinear + logmap0
            for t in range(NBLK):
                if l == 0:
                    nc.sync.dma_start(out=Vt(t), in_=x_d[t * P:(t + 1) * P, :])
                sc = work.tile([P, P], f32, tag="sq")
                nc.scalar.activation(out=sc[:], in_=Vt(t), func=AF.Square,
                                     accum_out=B0[:, t:t + 1])
            # B0 = sum v^2 per node
            if l == 0:
                _expmap_proj_chain(nc, B0[:], nbt[4:8], B1, B2)
                # B1 = s_enc, B2 = xn (= hn of encode)
                nc.vector.reciprocal(out=B3[:], in_=B2[:])      # 1/xn
            else:
                _sqrt_chain(nc, B0[:], B4, B2, B3)  # B2 = xn, B3 = 1/xn
            for t in range(NBLK):
                if l == 0:
                    nc.vector.tensor_scalar(out=Vt(t), in0=Vt(t),
                                            scalar1=B1[:, t:t + 1],
                                            scalar2=None, op0=ALU.mult)
                tp = psA.tile([P, P], f32, tag="tp")
                nc.tensor.transpose(out=tp[:], in_=Vt(t), identity=ident[:])
                vT = work.tile([P, P], f32, tag="vT")
                nc.vector.tensor_copy(out=vT[:], in_=tp[:])
                mxp = psB.tile([P, P], f32, tag="mxp")
                nc.tensor.matmul(out=mxp[:], lhsT=vT[:], rhs=wt_sb[l][:],
                                 start=True, stop=True)
                nc.vector.tensor_copy(out=Mt(t), in_=mxp[:])
                sc = work.tile([P, P], f32, tag="sq")
                nc.scalar.activation(out=sc[:], in_=mxp[:], func=AF.Square,
                                     accum_out=B4[:, t:t + 1])
            # chainB: S2P (scale for h) and HN (norm of h)
            _sqrt_chain(nc, B4[:], B5, B6, B7)          # B6=mxn, B7=1/mxn
            nc.vector.tensor_scalar(out=B5[:], in0=B2[:], scalar1=ACLIP,
                                    scalar2=None, op0=ALU.min)
            _artanh2(nc, B5[:], B8, B9, B5)             # B5 = 2*artanh(xn)
            nc.vector.tensor_tensor(out=B5[:], in0=B5[:], in1=B6[:],
                                    op=ALU.mult)
            nc.vector.tensor_tensor(out=B5[:], in0=B5[:], in1=B3[:],
                                    op=ALU.mult)        # = 2*r
            nc.vector.tensor_scalar(out=B5[:], in0=B5[:], scalar1=E2MAX,
                                    scalar2=None, op0=ALU.min)
            nc.scalar.activation(out=B5[:], in_=B5[:], func=AF.Exp)
            nc.vector.tensor_scalar(out=B5[:], in0=B5[:], scalar1=1.0,
                                    scalar2=None, op0=ALU.add)
            nc.vector.reciprocal(out=B5[:], in_=B5[:])
            nc.vector.tensor_scalar(out=B5[:], in0=B5[:], scalar1=-2.0,
                                    scalar2=1.0, op0=ALU.mult, op1=ALU.add)
            # B5 = th = tanh(r) >= 0
            nc.vector.tensor_scalar(out=B8[:], in0=B5[:], scalar1=1e-15,
                                    scalar2=None, op0=ALU.max)   # u
            nc.vector.tensor_scalar(out=B2[:], in0=B8[:], scalar1=MAXN,
                                    scalar2=None, op0=ALU.min)   # HN -> B2
            nc.vector.reciprocal(out=B8[:], in_=B8[:])
            nc.vector.tensor_tensor(out=B8[:], in0=B2[:], in1=B8[:],
                                    op=ALU.mult)                  # pf
            nc.vector.tensor_tensor(out=B5[:], in0=B5[:], in1=B7[:],
                                    op=ALU.mult)
            nc.vector.tensor_tensor(out=B5[:], in0=B5[:], in1=B8[:],
                                    op=ALU.mult)                  # S2P
            for t in range(NBLK):
                nc.vector.tensor_scalar(out=Vt(t), in0=Mt(t),
                                        scalar1=B5[:, t:t + 1], scalar2=None,
                                        op0=ALU.mult)             # V = h
                tm = work.tile([P, P], f32, tag="tm")
                nc.vector.tensor_tensor(out=tm[:], in0=Vt(t), in1=hb_sb[l][:],
                                        op=ALU.mult)
                nc.vector.reduce_sum(out=B0[:, t:t + 1], in_=tm[:],
                                     axis=mybir.AxisListType.X)   # xy
            # chainC: F1, F2 from xy (B0), HN (B2), y2
            y2 = float(y2s[l])
            nc.vector.tensor_tensor(out=B1[:], in0=B2[:], in1=B2[:],
                                    op=ALU.mult)                  # x2
            nc.vector.tensor_scalar(out=B6[:], in0=B0[:], scalar1=2.0,
                                    scalar2=1.0 + y2, op0=ALU.mult,
                                    op1=ALU.add)                  # a1
            nc.vector.tensor_scalar(out=B7[:], in0=B1[:], scalar1=-1.0,
                                    scalar2=1.0, op0=ALU.mult, op1=ALU.add)
            nc.vector.tensor_scalar(out=B8[:], in0=B7[:], scalar1=-y2,
                                    scalar2=None, op0=ALU.mult)
            nc.vector.tensor_tensor(out=B8[:], in0=B8[:], in1=B6[:],
                                    op=ALU.add)                   # den
            nc.vector.reciprocal(out=B8[:], in_=B8[:])
            nc.vector.tensor_tensor(out=B6[:], in0=B6[:], in1=B8[:],
                                    op=ALU.mult)                  # F1
            nc.vector.tensor_tensor(out=B7[:], in0=B7[:], in1=B8[:],
                                    op=ALU.mult)                  # F2
            for t in range(NBLK):
                t1 = work.tile([P, P], f32, tag="t1")
                nc.vector.tensor_scalar(out=t1[:], in0=Vt(t),
                                        scalar1=B6[:, t:t + 1], scalar2=None,
                                        op0=ALU.mult)
                t2 = work.tile([P, P], f32, tag="t2")
                nc.vector.tensor_scalar(out=t2[:], in0=hb_sb[l][:],
                                        scalar1=B7[:, t:t + 1], scalar2=None,
                                        op0=ALU.mult)
                nc.vector.tensor_tensor(out=Mt(t), in0=t1[:], in1=t2[:],
                                        op=ALU.add)               # M = h+b
                sc = work.tile([P, P], f32, tag="sq")
                nc.scalar.activation(out=sc[:], in_=Mt(t), func=AF.Square,
                                     accum_out=B0[:, t:t + 1])
            # chainD: S3 = 2*artanh(min(bn,maxn)) / bn   (apply *0.5 later)
            _sqrt_chain(nc, B0[:], B1, B2, B3)          # B2=bn, B3=1/bn
            nc.vector.tensor_scalar(out=B1[:], in0=B2[:], scalar1=MAXN,
                                    scalar2=None, op0=ALU.min)
            _artanh2(nc, B1[:], B8, B9, B1)
            nc.vector.tensor_tensor(out=B1[:], in0=B1[:], in1=B3[:],
                                    op=ALU.mult)                  # S3
            for t in range(NBLK):
                xt = work.tile([P, P], f32, tag="xt")
                nc.vector.tensor_scalar(out=xt[:], in0=Mt(t),
                                        scalar1=B1[:, t:t + 1], scalar2=0.5,
                                        op0=ALU.mult, op1=ALU.mult)
                nc.sync.dma_start(out=ag_in[l][t * P:(t + 1) * P, :],
                                  in_=xt[:])
            # ---------------- AllGather tangent vectors
            nc.gpsimd.collective_compute(
                "AllGather", ALU.bypass, replica_groups=rg,
                ins=[ag_in[l].opt()], outs=[xt_full[l].opt()])
            # ---------------- phase B: gather + weighted segment sum
            for b in range(NBLK):
                nbb = int(nb[b])
                co = int(coff[b])
                G = gpool.tile([P, nbmax * P], f32, tag="G")
                for j in range(nbb):
                    nc.gpsimd.indirect_dma_start(
                        out=G[:, j * P:(j + 1) * P], out_offset=None,
                        in_=xt_full[l][:, :],
                        in_offset=bass.IndirectOffsetOnAxis(
                            ap=midx_sb[:, co + j:co + j + 1], axis=0))
                agg = psC.tile([P, P], f32, tag="agg")
                for j in range(nbb):
                    sw = work.tile([P, P], f32, tag="sw")
                    nc.vector.tensor_scalar(
                        out=sw[:], in0=iota_f[:],
                        scalar1=mdst_sb[:, co + j:co + j + 1],
                        scalar2=mew_sb[:, co + j:co + j + 1],
                        op0=ALU.is_equal, op1=ALU.mult)
                    nc.tensor.matmul(out=agg[:], lhsT=sw[:],
                                     rhs=G[:, j * P:(j + 1) * P],
                                     start=(j == 0), stop=(j == nbb - 1))
                nc.vector.tensor_copy(out=Vt(b), in_=agg[:])
                sc = work.tile([P, P], f32, tag="sq")
                nc.scalar.activation(out=sc[:], in_=agg[:], func=AF.Square,
                                     accum_out=B0[:, b:b + 1])
            # chainE: S45H = 0.5 * s4 * (2*artanh(hn3)/hn3)
            _expmap_proj_chain(nc, B0[:], nbt[4:8], B1, B2)  # B1=s4, B2=hn3
            _artanh2(nc, B2[:], B8, B9, B6)                  # 2*artanh(hn3)
            nc.vector.reciprocal(out=B7[:], in_=B2[:])
            nc.vector.tensor_tensor(out=B6[:], in0=B6[:], in1=B7[:],
                                    op=ALU.mult)
            nc.vector.tensor_tensor(out=B6[:], in0=B6[:], in1=B1[:],
                                    op=ALU.mult)
            nc.vector.tensor_scalar(out=B6[:], in0=B6[:], scalar1=0.5,
                                    scalar2=None, op0=ALU.mult)  # S45H
            for b in range(NBLK):
                nc.scalar.activation(out=Mt(b), in_=Vt(b), func=AF.Relu,
                                     scale=B6[:, b:b + 1])
                sc = work.tile([P, P], f32, tag="sq")
                nc.scalar.activation(out=sc[:], in_=Mt(b), func=AF.Square,
                                     accum_out=B0[:, b:b + 1])
            # chainF: S6 (expmap0+proj of relu'd tangent)
            _expmap_proj_chain(nc, B0[:], nbt[4:8], B1, B2)  # B1 = s6
            for b in range(NBLK):
                if l == 0:
                    nc.vector.tensor_scalar(out=Vt(b), in0=Mt(b),
                                            scalar1=B1[:, b:b + 1],
                                            scalar2=None, op0=ALU.mult)
                else:
                    ot = work.tile([P, P], f32, tag="ot")
                    nc.vector.tensor_scalar(out=ot[:], in0=Mt(b),
                                            scalar1=B1[:, b:b + 1],
                                            scalar2=None, op0=ALU.mult)
                    nc.sync.dma_start(out=out_d[b * P:(b + 1) * P, :],
                                      in_=ot[:])
    return nc


# ----------------------------------------------------------------- host side
def _hyp_bias(b):
    b = b.astype(np.float32)
    n = max(float(np.linalg.norm(b)), 1e-15)
    hb = np.float32(np.tanh(n)) * b / np.float32(n)
    nn = float(np.linalg.norm(hb))
    if nn > MAXN:
        hb = hb / np.float32(nn) * np.float32(MAXN)
    return hb.astype(np.float32), float((hb.astype(np.float64) ** 2).sum())


def _prep_edges(src, dst, ew, N, NPAD, SHARD, NBLK, ncores):
    src = np.asarray(src).astype(np.int64)
    dst = np.asarray(dst).astype(np.int64)
    ew = np.asarray(ew).astype(np.float32)
    order = np.argsort(dst, kind="stable")
    s, d, w = src[order], dst[order], ew[order]
    core = d // SHARD
    blk = (d % SHARD) // P
    key = core * NBLK + blk
    cnt = np.bincount(key, minlength=ncores * NBLK).reshape(ncores, NBLK)
    nb = np.maximum(1, -(-cnt.max(axis=0) // P))        # per-block subtiles
    coff = np.zeros(NBLK + 1, np.int64)
    coff[1:] = np.cumsum(nb)
    CTOT = int(coff[-1])
    # group boundaries in the sorted edge list
    starts = np.zeros(ncores * NBLK + 1, np.int64)
    starts[1:] = np.cumsum(cnt.reshape(-1))
    midx = np.zeros((ncores, P, CTOT), np.int32)
    mdst = np.zeros((ncores, P, CTOT), np.float32)
    mew = np.zeros((ncores, P, CTOT), np.float32)
    drel = (d % P).astype(np.float32)
    for c in range(ncores):
        for b in range(NBLK):
            g0, g1 = starts[c * NBLK + b], starts[c * NBLK + b + 1]
            L = g1 - g0
            if L == 0:
                continue
            kk = np.arange(L)
            p = kk % P
            jj = coff[b] + kk // P
            midx[c, p, jj] = s[g0:g1]
            mdst[c, p, jj] = drel[g0:g1]
            mew[c, p, jj] = w[g0:g1]
    return nb, coff, CTOT, midx, mdst, mew


_CACHE = {}


def _get_program(NPAD, SHARD, NBLK, nb, coff, CTOT, y2s, ncores):
    key = (NPAD, tuple(int(v) for v in nb), tuple(round(v, 10) for v in y2s))
    if key in _CACHE:
        return _CACHE[key]
    nc = bacc.Bacc("TRN2", target_bir_lowering=False, debug=False,
                   enable_asserts=False, num_devices=ncores)
    build_program(nc, NPAD, SHARD, NBLK, nb, coff, CTOT, y2s, ncores)
    nc.compile()
    _CACHE[key] = nc
    return nc


def kernel(x, W1, b1, W2, b2, edge_weight, src, dst, _sim=False, _trace=False):
    x = np.asarray(x, np.float32)
    N = x.shape[0]
    ncores = 8
    SHARD = -(-N // (ncores * P)) * P
    NPAD = SHARD * ncores
    NBLK = SHARD // P
    xp = np.zeros((NPAD, P), np.float32)
    xp[:N] = x
    hb1, y21 = _hyp_bias(np.asarray(b1))
    hb2, y22 = _hyp_bias(np.asarray(b2))
    nb, coff, CTOT, midx, mdst, mew = _prep_edges(
        src, dst, edge_weight, N, NPAD, SHARD, NBLK, ncores)
    nc = _get_program(NPAD, SHARD, NBLK, nb, coff, CTOT, (y21, y22), ncores)
    wt1 = np.ascontiguousarray(np.asarray(W1, np.float32).T)
    wt2 = np.ascontiguousarray(np.asarray(W2, np.float32).T)
    hb1b = np.tile(hb1[None, :], (P, 1))
    hb2b = np.tile(hb2[None, :], (P, 1))
    in_maps = []
    for c in range(ncores):
        in_maps.append({
            "x": np.ascontiguousarray(xp[c * SHARD:(c + 1) * SHARD]),
            "wt1": wt1, "wt2": wt2, "hb1": hb1b, "hb2": hb2b,
            "midx": np.ascontiguousarray(midx[c]),
            "mdst": np.ascontiguousarray(mdst[c]),
            "mew": np.ascontiguousarray(mew[c]),
        })
    if _sim:
        from concourse.bass_interp import MultiCoreSim
        sim = MultiCoreSim(nc, num_cores=ncores, trace=False,
                           require_finite=False, require_nnan=False)
        cores = list(sim.cores.values())
        for c in range(ncores):
            for k, v in in_maps[c].items():
                cores[c].tensor(k)[:] = v
        sim.simulate(check_with_hw=False)
        outs = [np.array(cores[c].tensor("out")) for c in range(ncores)]
    else:
        res = run_bass_kernel_spmd(nc, in_maps, core_ids=list(range(ncores)),
                                   trace=_trace)
        if _trace:
            kernel._last_results = res
            print("exec_time_ns:", res.exec_time_ns)
        outs = [res.results[c]["out"] for c in range(ncores)]
    return np.concatenate(outs, axis=0)[:N].astype(np.float32)
